# revision 1
# baseline (speedup 1.0000x reference)
"""Multi-head attention (B=4, T=2048, D=1024, H=16) on 8 TRN2 NeuronCores.

Sharding: core c handles batch b = c//2 and head-half hh = c%2 (8 heads,
512 of the 1024 channel dims). Each core computes its half of the head
outputs and a row-sharded output projection, producing a partial
[T, D] output. Host unshard: out[b] = partial[2b] + partial[2b+1]
+ b_o + b_v @ w_o.T (the value-bias contribution commutes through
attention because softmax rows sum to 1).

v7: all-bf16 matmuls, ACT-paced softmax pipeline.
  PSUM: scores 2x[128,1024] double-buffered, av 2x[65,512] (also reused
  for the K=1 denominator-broadcast matmuls), proj 2x[128,512] shared by
  QKV projections and the output projection.
  - Softmax denominator rides row 64 of the av accumulators (ones column
    in V); a K=1 PE matmul broadcasts it across partitions, so the
    normalize chain (reciprocal, scale) never waits on a DMA.
  - t-block 0's attention is chunked by tk-quarters with SBUF
    accumulation, so it streams as each K/V projection t-block lands
    instead of stalling on the full K/V sweep.
  - Out-projection chains of the previous t-block and the next Q
    projection are woven into the attention emission as PE filler.
"""

from contextlib import ExitStack

import numpy as np
import ml_dtypes

import concourse.bass as bass
import concourse.mybir as mybir
import concourse.tile as tile
from concourse import bacc
from concourse.bass_utils import run_bass_kernel_spmd

B, T, D = 4, 2048, 1024
H = 16
DH = 64  # head dim
HALF = 512  # channels per core (8 heads)
N_CORES = 8

F32 = mybir.dt.float32
BF16 = mybir.dt.bfloat16

TB = 512  # t-block for moving operands
NTB = T // TB  # 4
KB = 128  # contraction block
NKB = D // KB  # 8
NJB = HALF // KB  # 4 j-blocks of the half
NTK = T // KB  # 16 tk blocks


def build_kernel():
    nc = bacc.Bacc(
        "TRN2", target_bir_lowering=False, debug=False, num_devices=N_CORES
    )
    xqT = nc.dram_tensor("xqT", [D, T], BF16, kind="ExternalInput").ap()
    xkT = nc.dram_tensor("xkT", [D, T], BF16, kind="ExternalInput").ap()
    xvT = nc.dram_tensor("xvT", [D, T], BF16, kind="ExternalInput").ap()
    wqT = nc.dram_tensor("wqT", [D, HALF], BF16, kind="ExternalInput").ap()
    wkT = nc.dram_tensor("wkT", [D, HALF], BF16, kind="ExternalInput").ap()
    wvT = nc.dram_tensor("wvT", [D, HALF], BF16, kind="ExternalInput").ap()
    woT = nc.dram_tensor("woT", [HALF, D], BF16, kind="ExternalInput").ap()
    bq = nc.dram_tensor("bq", [HALF, 1], F32, kind="ExternalInput").ap()
    bk = nc.dram_tensor("bk", [HALF, 1], F32, kind="ExternalInput").ap()
    ones_in = nc.dram_tensor("ones_in", [KB, H // 2], BF16, kind="ExternalInput").ap()
    ones_bc_in = nc.dram_tensor(
        "ones_bc_in", [DH + 1, DH], BF16, kind="ExternalInput"
    ).ap()
    partial = nc.dram_tensor("partial", [T, D], F32, kind="ExternalOutput").ap()

    with tile.TileContext(nc) as tc, ExitStack() as ctx:
        p_const = ctx.enter_context(tc.tile_pool(name="const", bufs=1))
        p_kt = ctx.enter_context(tc.tile_pool(name="kt", bufs=NJB * NTB))
        p_v = ctx.enter_context(tc.tile_pool(name="v", bufs=NTK))
        p_qt = ctx.enter_context(tc.tile_pool(name="qt", bufs=2 * NJB))
        p_xs = ctx.enter_context(tc.tile_pool(name="xs", bufs=42))
        p_ex = ctx.enter_context(tc.tile_pool(name="ex", bufs=6))
        p_ot = ctx.enter_context(tc.tile_pool(name="ot", bufs=2 * NJB))
        p_as = ctx.enter_context(tc.tile_pool(name="as", bufs=6))
        p_ac = ctx.enter_context(tc.tile_pool(name="ac", bufs=8))
        p_rc = ctx.enter_context(tc.tile_pool(name="rc", bufs=3))
        p_st = ctx.enter_context(tc.tile_pool(name="st", bufs=2))
        # PSUM: scores 2x2 banks + av 2x1 + proj/outproj 2x1 = 8 banks
        p_sc = ctx.enter_context(tc.tile_pool(name="sc", bufs=2, space="PSUM"))
        p_av = ctx.enter_context(tc.tile_pool(name="av", bufs=2, space="PSUM"))
        p_pj = ctx.enter_context(tc.tile_pool(name="pj", bufs=2, space="PSUM"))

        # ---- constants (w_o is emitted last: it is only needed once the
        # first out-projection runs, well after the first K/V chains) ----
        w_k = p_const.tile([KB, NKB, HALF], BF16, tag="wk")
        for kb in range(NKB):
            # per-kb chunks: the first K-projection matmuls start as soon as
            # the first 128 rows of the weight land, not after the full 1MB
            nc.sync.dma_start(w_k[:, kb, :], wkT[kb * KB : (kb + 1) * KB, :])
        b_k = p_const.tile([KB, NJB], F32, tag="bk")
        nc.sync.dma_start(b_k[:], bk.rearrange("(jb p) one -> p (jb one)", p=KB))
        # w_v/w_q/w_o and the ones constants are DMA'd later, interleaved
        # with the first projection chains, so they don't queue ahead of the
        # x-tile loads that gate the first matmuls
        ones8 = p_const.tile([KB, H // 2], BF16, tag="ones8")
        ones_bc = p_const.tile([DH + 1, DH], BF16, tag="onesbc")
        w_v = p_const.tile([KB, NKB, HALF], BF16, tag="wv")
        w_q = p_const.tile([KB, NKB, HALF], BF16, tag="wq")
        b_q = p_const.tile([KB, NJB], F32, tag="bq")
        w_o = p_const.tile([KB, NJB, D], BF16, tag="wo")

        def load_x_tiles(src, tb):
            """DMA one t-block of an input into 8 resident [128, 512] tiles."""
            xts = []
            for kb in range(NKB):
                xt = p_xs.tile([KB, TB], BF16, tag="xs")
                nc.sync.dma_start(
                    xt[:], src[kb * KB : (kb + 1) * KB, tb * TB : (tb + 1) * TB]
                )
                xts.append(xt)
            return xts

        # kt[jb][tb]: [128 (j), TB] tiles (separate tiles per t-block so
        # attention groups depend only on the t-blocks they read)
        kt_tiles = [
            [p_kt.tile([KB, TB], BF16, tag="kt", name=f"kt{j}_{tb}") for tb in range(NTB)]
            for j in range(NJB)
        ]
        v_tiles = [
            p_v.tile([KB, H // 2, DH + 1], BF16, tag="v", name=f"v{j}")
            for j in range(NTK)
        ]

        def kv_proj_chains(tb):
            """K^T + V projection chains for one t-block, as thunks."""
            xk_tiles, xv_tiles = [], []

            def k_chain(jb):
                def emit():
                    if not xk_tiles:
                        xk_tiles.extend(load_x_tiles(xkT, tb))
                    ps = p_pj.tile([KB, TB], F32, tag="pj")
                    for kb in range(NKB):
                        nc.tensor.matmul(
                            ps[:],
                            w_k[:, kb, jb * KB : (jb + 1) * KB],
                            xk_tiles[kb][:],
                            start=(kb == 0),
                            stop=(kb == NKB - 1),
                        )
                    nc.vector.tensor_scalar_add(
                        kt_tiles[jb][tb][:], ps[:], b_k[:, jb : jb + 1]
                    )

                return emit

            def v_chain(ts):
                def emit():
                    if not xv_tiles:
                        for u in range(4):
                            nc.sync.dma_start(
                                v_tiles[tb * 4 + u][:, :, DH : DH + 1],
                                ones8[:, :, None],
                            )
                        xv_tiles.extend(load_x_tiles(xvT, tb))
                    ps = p_pj.tile([KB, TB], F32, tag="pj")
                    for kb in range(NKB):
                        nc.tensor.matmul(
                            ps[:],
                            xv_tiles[kb][:, ts * KB : (ts + 1) * KB],
                            w_v[:, kb, :],
                            start=(kb == 0),
                            stop=(kb == NKB - 1),
                        )
                    nc.vector.tensor_copy(
                        v_tiles[tb * 4 + ts][:, :, 0:DH],
                        ps[:].rearrange("p (h d) -> p h d", d=DH),
                    )

                return emit

            # K chains before V chains: the shared x-tile pool releases the
            # xk residents before the xv loads need slots
            return [k_chain(u) for u in range(4)] + [v_chain(u) for u in range(4)]

        def kv_proj_chains_jit(tb):
            """Same chains split by first consumer: (jb0 K-chain + V chains,
            which head-pair 0's sweep needs) and (K jb1-3, deferred until
            head pairs 1-3 reach this t-block)."""
            ch = kv_proj_chains(tb)
            return [ch[0]] + ch[4:], ch[1:4]

        def q_proj(tq):
            qt_tiles = [
                p_qt.tile([KB, TB], BF16, tag="qt", name=f"qt{j}") for j in range(NJB)
            ]
            xts = load_x_tiles(xqT, tq)
            for jb in range(NJB):
                ps = p_pj.tile([KB, TB], F32, tag="pj")
                for kb in range(NKB):
                    nc.tensor.matmul(
                        ps[:],
                        w_q[:, kb, jb * KB : (jb + 1) * KB],
                        xts[kb][:],
                        start=(kb == 0),
                        stop=(kb == NKB - 1),
                    )
                nc.vector.tensor_scalar_add(
                    qt_tiles[jb][:], ps[:], b_q[:, jb : jb + 1]
                )
            return qt_tiles

        def normalize(jp, i, src, ot_tiles):
            """src: [DH+1, TB] f32 (SBUF) accumulated head output; row DH is
            the softmax denominator. PE broadcasts it across partitions via a
            K=1 matmul; reciprocal+scale on DVE; result -> ot pair tile."""
            db = p_as.tile([DH + 1, TB], BF16, tag="db")
            nc.vector.tensor_copy(db[DH : DH + 1, :], src[DH : DH + 1, :])
            dbc = p_av.tile([DH, TB], F32, tag="av", name="dbc")
            nc.tensor.matmul(
                dbc[:],
                ones_bc[DH : DH + 1, :],
                db[DH : DH + 1, :],
                start=True,
                stop=True,
            )
            rc2 = p_rc.tile([DH, TB], F32, tag="rc2")
            nc.vector.reciprocal_approx_fast(rc2[:], dbc[:])
            if i == 0:
                nc.vector.tensor_mul(ot_tiles[jp][0:DH, :], src[0:DH, :], rc2[:])
            else:
                # DVE can't shift partitions; stage then DMA into rows 64:128
                stg = p_rc.tile([DH, TB], BF16, tag="stg")
                nc.vector.tensor_mul(stg[:], src[0:DH, :], rc2[:])
                nc.sync.dma_start(ot_tiles[jp][DH : 2 * DH, :], stg[:])

        def attention(
            qt_tiles,
            chunks,
            chunk_pre=None,
            filler=(),
            jp_post=None,
            group_filler=(),
            skip_groups=0,
        ):
            """One t-block of attention over tk chunks; returns ot pair-tiles.

            chunks: list of tk-index lists. Single chunk: accumulate in psum
            and normalize straight from it. Multiple chunks: spill/add each
            chunk into an SBUF accumulator (so attention streams while later
            K/V t-blocks are still being projected).
            chunk_pre: {chunk_idx: [thunks]} emitted before that chunk.
            filler: thunks woven in after each head pair's normalize.
            jp_post: {jp: [thunks]} emitted after that pair's normalize.
            """
            chunk_pre = chunk_pre or {}
            jp_post = jp_post or {}
            filler = list(filler)
            group_filler = list(group_filler)
            gidx = 0
            chunked = len(chunks) > 1
            ot_tiles = [
                p_ot.tile([KB, TB], BF16, tag="ot", name=f"ot{j}") for j in range(NJB)
            ]
            acc = {}
            if chunked:
                for jp in range(NJB):
                    for i in range(2):
                        acc[jp, i] = p_ac.tile(
                            [DH + 1, TB], F32, tag="ac", name=f"ac{jp}_{i}"
                        )

            for ci, chunk in enumerate(chunks):
                for thunk in chunk_pre.get(ci, []):
                    thunk()
                last_chunk = ci == len(chunks) - 1
                for jp in range(NJB):  # head pair (2*jp, 2*jp+1)
                    avs = [
                        p_av.tile([DH + 1, TB], F32, tag="av", name=f"av{i}")
                        for i in range(2)
                    ]
                    for tk in chunk:
                        sc = p_sc.tile([KB, 2 * TB], F32, tag="sc")
                        # scores: the two matmuls hit row groups 0/64 and run
                        # concurrently in the PE array
                        for i in range(2):
                            nc.tensor.matmul(
                                sc[:, i * TB : (i + 1) * TB],
                                kt_tiles[jp][tk // 4][
                                    i * DH : (i + 1) * DH,
                                    (tk % 4) * KB : (tk % 4 + 1) * KB,
                                ],
                                qt_tiles[jp][i * DH : (i + 1) * DH, :],
                                start=True,
                                stop=True,
                            )
                        ex = p_ex.tile([KB, 2 * TB], BF16, tag="ex")
                        nc.scalar.activation(
                            ex[:], sc[:], mybir.ActivationFunctionType.Exp, scale=0.125
                        )
                        for i in range(2):
                            nc.tensor.matmul(
                                avs[i][:],
                                v_tiles[tk][:, 2 * jp + i, :],
                                ex[:, i * TB : (i + 1) * TB],
                                start=(tk == chunk[0]),
                                stop=(tk == chunk[-1]),
                            )
                        gidx += 1
                        if group_filler and gidx > skip_groups:
                            group_filler.pop(0)()
                    if chunked:
                        for i in range(2):
                            if ci == 0:
                                nc.vector.tensor_copy(acc[jp, i][:], avs[i][:])
                            else:
                                nc.vector.tensor_add(
                                    acc[jp, i][:], acc[jp, i][:], avs[i][:]
                                )
                        if last_chunk:
                            for i in (1, 0):
                                normalize(jp, i, acc[jp, i], ot_tiles)
                    else:
                        av_s = []
                        for i in range(2):
                            a = p_as.tile([DH + 1, TB], F32, tag="as")
                            nc.vector.tensor_copy(a[:], avs[i][:])
                            av_s.append(a)
                        for i in (1, 0):
                            normalize(jp, i, av_s[i], ot_tiles)
                    if last_chunk:
                        for thunk in jp_post.get(jp, []):
                            thunk()
                        take = (
                            len(filler) // (NJB - jp)
                            if jp < NJB - 1
                            else len(filler)
                        )
                        for _ in range(take):
                            filler.pop(0)()
            return ot_tiles

        def out_proj_chains(tq, ot_tiles):
            def chain(nb, ts):
                def emit():
                    po = p_pj.tile([KB, TB], F32, tag="pj")
                    for jp in range(NJB):
                        nc.tensor.matmul(
                            po[:],
                            ot_tiles[jp][:, ts * KB : (ts + 1) * KB],
                            w_o[:, jp, nb * TB : (nb + 1) * TB],
                            start=(jp == 0),
                            stop=(jp == NJB - 1),
                        )
                    st = p_st.tile([KB, TB], F32, tag="st")
                    nc.vector.tensor_copy(st[:], po[:])
                    nc.sync.dma_start(
                        partial[
                            tq * TB + ts * KB : tq * TB + (ts + 1) * KB,
                            nb * TB : (nb + 1) * TB,
                        ],
                        st[:],
                    )

                return emit

            return [chain(nb, ts) for nb in range(2) for ts in range(4)]

        # ---- emission ----
        chains0 = kv_proj_chains(0)
        for thunk in chains0[:4]:  # K chains of t-block 0
            thunk()
        for kb in range(NKB):
            nc.sync.dma_start(w_v[:, kb, :], wvT[kb * KB : (kb + 1) * KB, :])
        nc.sync.dma_start(ones8[:], ones_in[:])
        nc.sync.dma_start(ones_bc[:], ones_bc_in[:])
        for thunk in chains0[4:]:  # V chains of t-block 0
            thunk()
        for kb in range(NKB):
            nc.sync.dma_start(w_q[:, kb, :], wqT[kb * KB : (kb + 1) * KB, :])
        nc.sync.dma_start(b_q[:], bq.rearrange("(jb p) one -> p (jb one)", p=KB))
        qt = q_proj(0)

        qt_next = []

        def q_thunk(tq):
            def emit():
                qt_next.append(q_proj(tq))

            return emit

        # K/V projections for t-blocks 1-3 are woven into tq0's attention
        # group loop (1 chain per group): they run under exp latency instead
        # of as a serial prefix. Chains are ordered just-in-time: what head
        # pair 0's tk sweep needs comes first; the K jb1-3 chains follow in
        # the order head pairs 1-3 consume them.
        front, backs = [], [[], [], []]
        for tb in range(1, NTB):
            f, b = kv_proj_chains_jit(tb)
            front.extend(f)
            for j in range(3):
                backs[j].append(b[j])
        late_chains = front + backs[0] + backs[1] + backs[2]
        late_chains.append(
            lambda: nc.sync.dma_start(
                w_o[:], woT.rearrange("(jb p) n -> p jb n", p=KB)
            )
        )

        pending = []  # out-projection chains of the previous t-block
        for tq in range(NTB):
            qt_next.clear()
            ot = attention(
                qt,
                [list(range(NTK))],
                filler=pending,
                jp_post={2: [q_thunk(tq + 1)]} if tq + 1 < NTB else {},
                group_filler=late_chains if tq == 0 else (),
            )
            if tq + 1 < NTB:
                qt = qt_next[0]
            pending = out_proj_chains(tq, ot)
        for c in pending:
            c()

    nc.compile()
    return nc


def kernel(**inputs: np.ndarray) -> np.ndarray:
    query = np.asarray(inputs["query"], dtype=np.float32)
    key = np.asarray(inputs["key"], dtype=np.float32)
    value = np.asarray(inputs["value"], dtype=np.float32)
    w_q = np.asarray(inputs["w_q"], dtype=np.float32)
    b_q = np.asarray(inputs["b_q"], dtype=np.float32)
    w_k = np.asarray(inputs["w_k"], dtype=np.float32)
    b_k = np.asarray(inputs["b_k"], dtype=np.float32)
    w_v = np.asarray(inputs["w_v"], dtype=np.float32)
    b_v = np.asarray(inputs["b_v"], dtype=np.float32)
    w_o = np.asarray(inputs["w_o"], dtype=np.float32)
    b_o = np.asarray(inputs["b_o"], dtype=np.float32)

    nc = build_kernel()

    bf = ml_dtypes.bfloat16
    in_maps = []
    for c in range(N_CORES):
        b = c // 2
        hh = c % 2
        sl = slice(hh * HALF, (hh + 1) * HALF)
        in_maps.append(
            {
                "xqT": np.ascontiguousarray(query[b].T.astype(bf)),
                "xkT": np.ascontiguousarray(key[b].T.astype(bf)),
                "xvT": np.ascontiguousarray(value[b].T.astype(bf)),
                "wqT": np.ascontiguousarray(w_q[sl, :].T.astype(bf)),
                "wkT": np.ascontiguousarray(w_k[sl, :].T.astype(bf)),
                "wvT": np.ascontiguousarray(w_v[sl, :].T.astype(bf)),
                "woT": np.ascontiguousarray(w_o[:, sl].T.astype(bf)),
                "bq": np.ascontiguousarray(b_q[sl].reshape(HALF, 1)),
                "bk": np.ascontiguousarray(b_k[sl].reshape(HALF, 1)),
                "ones_in": np.ones((KB, H // 2), dtype=bf),
                "ones_bc_in": np.ones((DH + 1, DH), dtype=bf),
            }
        )

    res = run_bass_kernel_spmd(nc, in_maps, core_ids=list(range(N_CORES)))

    const_row = (b_v[None, :] @ w_o.T + b_o[None, :]).astype(np.float32)
    out = np.empty((B, T, D), dtype=np.float32)
    for b in range(B):
        out[b] = res.results[2 * b]["partial"] + res.results[2 * b + 1]["partial"]
        out[b] += const_row
    return out



# revision 10
# speedup vs baseline: 1.1728x; 1.1728x over previous
"""Multi-head attention (B=4, T=2048, D=1024, H=16) on 8 TRN2 NeuronCores.

Sharding: core c handles batch b = c//2 and head-half hh = c%2 (8 heads,
512 of the 1024 channel dims). Each core computes its half of the head
outputs and a row-sharded output projection, producing a partial
[T, D] output. Host unshard: out[b] = partial[2b] + partial[2b+1]
+ b_o + b_v @ w_o.T (the value-bias contribution commutes through
attention because softmax rows sum to 1).

v8: flat software-pipelined emission, ACT-saturating schedule.
  - The ACT exp stream (256 x [128,1024] activations, ~1.3us each) is the
    critical engine; the driver emits one attention step per exp and
    weaves all projection work into PE slack between steps.
  - Startup critical path: only xk/xq of t-block 0 plus the jb0 weight
    slices (~2.5MB) gate the first exp. Host pre-tiles inputs so every
    DMA source is a contiguous block.
  - tq0's attention is chunked by key-t-block (SBUF accumulation) so it
    streams while K/V of t-blocks 1-3 are still being projected.
  - Scores pairs run concurrently in PE row groups 0/64; softmax
    denominator rides row 64 of the AV accumulators (ones column in V);
    a K=1 PE matmul broadcasts it across partitions.
"""

from contextlib import ExitStack

import numpy as np
import ml_dtypes

import concourse.bass as bass
import concourse.mybir as mybir
import concourse.tile as tile
from concourse import bacc
from concourse.bass_utils import run_bass_kernel_spmd

B, T, D = 4, 2048, 1024
H = 16
DH = 64  # head dim
HALF = 512  # channels per core (8 heads)
N_CORES = 8

F32 = mybir.dt.float32
BF16 = mybir.dt.bfloat16

TB = 512  # t-block for moving operands
NTB = T // TB  # 4
KB = 128  # contraction block
NKB = D // KB  # 8
NJB = HALF // KB  # 4 j-blocks of the half
NTK = T // KB  # 16 tk blocks


class Step:
    __slots__ = ("tq", "jp", "tk", "seg_first", "seg_last", "final", "sc", "ex", "seg")

    def __init__(self, tq, jp, tk, seg_first, seg_last, final):
        self.tq = tq
        self.jp = jp
        self.tk = tk
        self.seg_first = seg_first  # first step of an av accumulation segment
        self.seg_last = seg_last  # last step of an av accumulation segment
        self.final = final  # last segment of this (tq, jp): normalize after
        self.sc = None
        self.ex = None
        self.seg = None


def build_kernel():
    nc = bacc.Bacc(
        "TRN2", target_bir_lowering=False, debug=False, num_devices=N_CORES
    )
    # pre-tiled inputs: x*[kb][tb] -> [128, 512] contiguous blocks
    xq = nc.dram_tensor("xq", [NKB * NTB * KB, TB], BF16, kind="ExternalInput").ap()
    xk = nc.dram_tensor("xk", [NKB * NTB * KB, TB], BF16, kind="ExternalInput").ap()
    xv = nc.dram_tensor("xv", [NKB * NTB * KB, TB], BF16, kind="ExternalInput").ap()
    # wq/wk tiled [jb][kb] -> [128, 128] contiguous blocks
    wq = nc.dram_tensor("wq", [NJB * NKB * KB, KB], BF16, kind="ExternalInput").ap()
    wk = nc.dram_tensor("wk", [NJB * NKB * KB, KB], BF16, kind="ExternalInput").ap()
    # wv rows contiguous per kb block; wo rows contiguous per jb block
    wv = nc.dram_tensor("wv", [NKB * KB, HALF], BF16, kind="ExternalInput").ap()
    wo = nc.dram_tensor("wo", [HALF, D], BF16, kind="ExternalInput").ap()
    bq = nc.dram_tensor("bq", [HALF, 1], F32, kind="ExternalInput").ap()
    bk = nc.dram_tensor("bk", [HALF, 1], F32, kind="ExternalInput").ap()
    ones_in = nc.dram_tensor("ones_in", [KB, H // 2], BF16, kind="ExternalInput").ap()
    ones_bc_in = nc.dram_tensor(
        "ones_bc_in", [DH + 1, DH], BF16, kind="ExternalInput"
    ).ap()
    partial = nc.dram_tensor("partial", [T, D], F32, kind="ExternalOutput").ap()

    with tile.TileContext(nc) as tc, ExitStack() as ctx:
        p_const = ctx.enter_context(tc.tile_pool(name="const", bufs=1))
        p_kt = ctx.enter_context(tc.tile_pool(name="kt", bufs=NJB * NTB))
        p_v = ctx.enter_context(tc.tile_pool(name="v", bufs=NTK))
        p_qt = ctx.enter_context(tc.tile_pool(name="qt", bufs=2 * NJB))
        p_xs = ctx.enter_context(tc.tile_pool(name="xs", bufs=32))
        p_ex = ctx.enter_context(tc.tile_pool(name="ex", bufs=8))
        p_ot = ctx.enter_context(tc.tile_pool(name="ot", bufs=2 * NJB))
        p_as = ctx.enter_context(tc.tile_pool(name="as", bufs=6))
        p_ac = ctx.enter_context(tc.tile_pool(name="ac", bufs=8))
        p_rc = ctx.enter_context(tc.tile_pool(name="rc", bufs=3))
        p_st = ctx.enter_context(tc.tile_pool(name="st", bufs=2))
        # PSUM: scores 2x[128,1024] (4 banks) + av 2x[65,512] + pj 2x[128,512]
        p_sc = ctx.enter_context(tc.tile_pool(name="sc", bufs=2, space="PSUM"))
        p_av = ctx.enter_context(tc.tile_pool(name="av", bufs=2, space="PSUM"))
        p_pj = ctx.enter_context(tc.tile_pool(name="pj", bufs=2, space="PSUM"))

        # ---- persistent SBUF tiles ----
        w_k = p_const.tile([KB, NKB, HALF], BF16, tag="wk")
        w_q = p_const.tile([KB, NKB, HALF], BF16, tag="wq")
        w_v = p_const.tile([KB, NKB, HALF], BF16, tag="wv")
        w_o = p_const.tile([KB, NJB, D], BF16, tag="wo")
        b_k = p_const.tile([KB, NJB], F32, tag="bk")
        b_q = p_const.tile([KB, NJB], F32, tag="bq")
        ones8 = p_const.tile([KB, H // 2], BF16, tag="ones8")
        ones_bc = p_const.tile([DH + 1, DH], BF16, tag="onesbc")
        warm = p_const.tile([1, 8], F32, tag="warm")
        warm_o = p_const.tile([1, 8], BF16, tag="warmo")

        kt_tiles = [
            [p_kt.tile([KB, TB], BF16, tag="kt", name=f"kt{j}_{tb}") for tb in range(NTB)]
            for j in range(NJB)
        ]
        v_tiles = [
            p_v.tile([KB, H // 2, DH + 1], BF16, tag="v", name=f"v{j}")
            for j in range(NTK)
        ]

        # ---- DMA emitters ----
        def dma_w_jb(dst, src, jb):
            # one jb slice of wq/wk: 8 kb blocks of [128, 128], contiguous src
            for kb in range(NKB):
                nc.sync.dma_start(
                    dst[:, kb, jb * KB : (jb + 1) * KB],
                    src[(jb * NKB + kb) * KB : (jb * NKB + kb + 1) * KB, :],
                )

        def load_x_tiles(src, tb):
            xts = []
            for kb in range(NKB):
                xt = p_xs.tile([KB, TB], BF16, tag="xs")
                nc.sync.dma_start(
                    xt[:], src[(kb * NTB + tb) * KB : (kb * NTB + tb + 1) * KB, :]
                )
                xts.append(xt)
            return xts

        # ---- projection chain emitters (each returns two half-chain thunks
        # so the driver can spread chains across pipeline steps) ----
        xk_tiles = {}  # tb -> tiles (shared by the 4 jb chains)
        xq_tiles = {}
        xv_tiles = {}
        HC = NKB // 2  # MMs per half-chain

        def k_parts(tb, jb):
            st = {}

            def a():
                if tb not in xk_tiles:
                    xk_tiles[tb] = load_x_tiles(xk, tb)
                st["ps"] = p_pj.tile([KB, TB], F32, tag="pj", name="ps")
                for kb in range(HC):
                    nc.tensor.matmul(
                        st["ps"][:],
                        w_k[:, kb, jb * KB : (jb + 1) * KB],
                        xk_tiles[tb][kb][:],
                        start=(kb == 0),
                        stop=False,
                    )

            def b():
                for kb in range(HC, NKB):
                    nc.tensor.matmul(
                        st["ps"][:],
                        w_k[:, kb, jb * KB : (jb + 1) * KB],
                        xk_tiles[tb][kb][:],
                        start=False,
                        stop=(kb == NKB - 1),
                    )
                nc.vector.tensor_scalar_add(
                    kt_tiles[jb][tb][:], st["ps"][:], b_k[:, jb : jb + 1]
                )
                if jb == NJB - 1:
                    del xk_tiles[tb]

            return a, b

        def v_parts(tb, ts):
            st = {}

            def a():
                if tb not in xv_tiles:
                    for u in range(4):
                        nc.sync.dma_start(
                            v_tiles[tb * 4 + u][:, :, DH : DH + 1],
                            ones8[:, :, None],
                        )
                    xv_tiles[tb] = load_x_tiles(xv, tb)
                st["ps"] = p_pj.tile([KB, TB], F32, tag="pj", name="ps")
                for kb in range(HC):
                    nc.tensor.matmul(
                        st["ps"][:],
                        xv_tiles[tb][kb][:, ts * KB : (ts + 1) * KB],
                        w_v[:, kb, :],
                        start=(kb == 0),
                        stop=False,
                    )

            def b():
                for kb in range(HC, NKB):
                    nc.tensor.matmul(
                        st["ps"][:],
                        xv_tiles[tb][kb][:, ts * KB : (ts + 1) * KB],
                        w_v[:, kb, :],
                        start=False,
                        stop=(kb == NKB - 1),
                    )
                nc.vector.tensor_copy(
                    v_tiles[tb * 4 + ts][:, :, 0:DH],
                    st["ps"][:].rearrange("p (h d) -> p h d", d=DH),
                )
                if ts == 3:
                    del xv_tiles[tb]

            return a, b

        qt_gen = {}  # tq -> list of qt tiles

        def q_parts(tq, jb):
            st = {}

            def a():
                if tq not in qt_gen:
                    qt_gen[tq] = [None] * NJB
                    xq_tiles[tq] = load_x_tiles(xq, tq)
                st["ps"] = p_pj.tile([KB, TB], F32, tag="pj", name="ps")
                for kb in range(HC):
                    nc.tensor.matmul(
                        st["ps"][:],
                        w_q[:, kb, jb * KB : (jb + 1) * KB],
                        xq_tiles[tq][kb][:],
                        start=(kb == 0),
                        stop=False,
                    )

            def b():
                for kb in range(HC, NKB):
                    nc.tensor.matmul(
                        st["ps"][:],
                        w_q[:, kb, jb * KB : (jb + 1) * KB],
                        xq_tiles[tq][kb][:],
                        start=False,
                        stop=(kb == NKB - 1),
                    )
                qt = p_qt.tile([KB, TB], BF16, tag="qt", name=f"qt{jb}")
                nc.vector.tensor_scalar_add(qt[:], st["ps"][:], b_q[:, jb : jb + 1])
                qt_gen[tq][jb] = qt
                if jb == NJB - 1:
                    del xq_tiles[tq]

            return a, b

        # ---- attention step emitters ----
        def emit_sc(d):
            d.sc = p_sc.tile([KB, 2 * TB], F32, tag="sc", name="sc")
            qt = qt_gen[d.tq]
            for i in range(2):
                nc.tensor.matmul(
                    d.sc[:, i * TB : (i + 1) * TB],
                    kt_tiles[d.jp][d.tk // 4][
                        i * DH : (i + 1) * DH,
                        (d.tk % 4) * KB : (d.tk % 4 + 1) * KB,
                    ],
                    qt[d.jp][i * DH : (i + 1) * DH, :],
                    start=True,
                    stop=True,
                )

        def emit_exp(d):
            d.ex = p_ex.tile([KB, 2 * TB], BF16, tag="ex", name="ex")
            nc.scalar.activation(
                d.ex[:], d.sc[:], mybir.ActivationFunctionType.Exp, scale=0.125
            )

        seg_avs = {}  # (tq, jp) -> current av psum pair

        def emit_av(d):
            if d.seg_first:
                seg_avs[d.tq, d.jp] = [
                    p_av.tile([DH + 1, TB], F32, tag="av", name=f"av{i}")
                    for i in range(2)
                ]
            avs = seg_avs[d.tq, d.jp]
            for i in range(2):
                nc.tensor.matmul(
                    avs[i][:],
                    v_tiles[d.tk][:, 2 * d.jp + i, :],
                    d.ex[:, i * TB : (i + 1) * TB],
                    start=d.seg_first,
                    stop=d.seg_last,
                )

        acc = {}  # (tq, jp, i) -> sbuf accumulator (chunked tq only)
        ot_gen = {}  # tq -> ot tiles

        def emit_spill(d, first_chunk):
            """Chunked path: move/add av psum pair into SBUF accumulators."""
            avs = seg_avs.pop((d.tq, d.jp))
            for i in range(2):
                if first_chunk:
                    acc[d.tq, d.jp, i] = p_ac.tile(
                        [DH + 1, TB], F32, tag="ac", name=f"ac{d.jp}_{i}"
                    )
                    nc.vector.tensor_copy(acc[d.tq, d.jp, i][:], avs[i][:])
                else:
                    nc.vector.tensor_add(
                        acc[d.tq, d.jp, i][:], acc[d.tq, d.jp, i][:], avs[i][:]
                    )

        def normalize(tq, jp, i, src):
            """src: [65, TB] f32 head-pair output; row 64 = denominator."""
            ot_tiles = ot_gen[tq]
            db = p_as.tile([DH + 1, TB], BF16, tag="db")
            nc.vector.tensor_copy(db[DH : DH + 1, :], src[DH : DH + 1, :])
            dbc = p_av.tile([DH, TB], F32, tag="av", name="dbc")
            nc.tensor.matmul(
                dbc[:],
                ones_bc[DH : DH + 1, :],
                db[DH : DH + 1, :],
                start=True,
                stop=True,
            )
            rc2 = p_rc.tile([DH, TB], F32, tag="rc2")
            nc.vector.reciprocal_approx_fast(rc2[:], dbc[:])
            if i == 0:
                nc.vector.tensor_mul(ot_tiles[jp][0:DH, :], src[0:DH, :], rc2[:])
            else:
                stg = p_rc.tile([DH, TB], BF16, tag="stg")
                nc.vector.tensor_mul(stg[:], src[0:DH, :], rc2[:])
                nc.sync.dma_start(ot_tiles[jp][DH : 2 * DH, :], stg[:])

        def emit_norm(d):
            """Final segment of (tq, jp): drain av psum / acc and normalize."""
            if d.tq == 0:
                emit_spill(d, False)
                srcs = [acc[d.tq, d.jp, i] for i in range(2)]
            else:
                avs = seg_avs.pop((d.tq, d.jp))
                srcs = []
                for i in range(2):
                    a = p_as.tile([DH + 1, TB], F32, tag="as")
                    nc.vector.tensor_copy(a[:], avs[i][:])
                    srcs.append(a)
            for i in (1, 0):
                normalize(d.tq, d.jp, i, srcs[i])

        def out_chain(tq, nb, ts):
            def emit():
                ot_tiles = ot_gen[tq]
                po = p_pj.tile([KB, TB], F32, tag="pj")
                for jp in range(NJB):
                    nc.tensor.matmul(
                        po[:],
                        ot_tiles[jp][:, ts * KB : (ts + 1) * KB],
                        w_o[:, jp, nb * TB : (nb + 1) * TB],
                        start=(jp == 0),
                        stop=(jp == NJB - 1),
                    )
                st = p_st.tile([KB, TB], F32, tag="st")
                nc.vector.tensor_copy(st[:], po[:])
                nc.sync.dma_start(
                    partial[
                        tq * TB + ts * KB : tq * TB + (ts + 1) * KB,
                        nb * TB : (nb + 1) * TB,
                    ],
                    st[:],
                )

            return emit

        # ---- build the global step list ----
        steps = []
        for tq in range(NTB):
            if tq == 0:
                for ci in range(4):
                    for jp in range(NJB):
                        for u in range(4):
                            steps.append(
                                Step(tq, jp, ci * 4 + u, u == 0, u == 3, ci == 3)
                            )
            else:
                for jp in range(NJB):
                    for u in range(NTK):
                        steps.append(Step(tq, jp, u, u == 0, u == NTK - 1, True))

        # ---- per-step filler plan: fillers[s] emitted between exp(s) and
        # sc(s+1).  Deadlines: a chain feeding sc(s+1) must be in fillers[<=s];
        # one feeding av(s-LAG) must be in fillers[<=s] as well (av comes last).
        AV_LAG = 6
        fillers = [[] for _ in steps]

        def place(s, thunk):
            fillers[min(max(s, 0), len(steps) - 1)].append(thunk)

        # tq0: K/Q chains jb1-3 (sc(4*jb) lookahead at step 4*jb-1)
        for j, s0 in [(1, 0), (2, 4), (3, 8)]:
            ka, kb_ = k_parts(0, j)
            qa, qb = q_parts(0, j)
            place(s0, ka)
            place(s0 + 1, kb_)
            place(s0 + 2, qa)
            place(s0 + 3, qb)
        # V tb0: av(u) runs at step u+AV_LAG
        for u in range(4):
            va, vb = v_parts(0, u)
            place(2 + 2 * u, va)
            place(3 + 2 * u, vb)
        # K/V of tb 1-3: kt[jp][tb] first read by sc at step 16*tb+4*jp-1;
        # v_tiles[4*tb+u] first read by av at step 16*tb+u+AV_LAG.
        for tb in range(1, NTB):
            for j in range(NJB):
                ka, kb_ = k_parts(tb, j)
                place(16 * tb - 6 + 2 * j, ka)
                place(16 * tb - 5 + 2 * j, kb_)
            for u in range(4):
                va, vb = v_parts(tb, u)
                place(16 * tb + 2 + 2 * u, va)
                place(16 * tb + 3 + 2 * u, vb)
        place(30, lambda: nc.sync.dma_start(
            w_o[:], wo.rearrange("(jb p) n -> p jb n", p=KB)))
        # q_proj(tq+1) near the end of tq; out_proj(tq-1) spread across tq.
        for tq in range(NTB):
            base = tq * 64
            if tq + 1 < NTB:
                for jb in range(NJB):
                    qa, qb = q_parts(tq + 1, jb)
                    place(base + 56 + 2 * jb, qa)
                    place(base + 57 + 2 * jb, qb)
            if tq > 0:
                chains = [out_chain(tq - 1, nb, ts) for nb in range(2) for ts in range(4)]
                for idx, ch in enumerate(chains):
                    place(base + 6 + 4 * idx, ch)

        # ---- emission ----
        # ACT table pre-warm: a tiny exp long before the first real one
        nc.sync.dma_start(warm[:, 0:1], bq[0:1, 0:1])
        nc.scalar.activation(
            warm_o[:, 0:1], warm[:, 0:1], mybir.ActivationFunctionType.Exp, scale=1.0
        )

        # startup DMA critical path
        for kb in range(NKB):
            nc.sync.dma_start(
                w_k[:, kb, 0:KB], wk[kb * KB : (kb + 1) * KB, :]
            )  # jb0 slice
        nc.sync.dma_start(b_k[:], bk.rearrange("(jb p) one -> p (jb one)", p=KB))
        ka0, kb0 = k_parts(0, 0)
        ka0()
        kb0()
        for kb in range(NKB):
            nc.sync.dma_start(w_q[:, kb, 0:KB], wq[kb * KB : (kb + 1) * KB, :])
        nc.sync.dma_start(b_q[:], bq.rearrange("(jb p) one -> p (jb one)", p=KB))
        qa0, qb0 = q_parts(0, 0)
        qa0()
        qb0()
        # next DMAs: remaining wk/wq jb slices, then V-path constants
        for jb in range(1, NJB):
            dma_w_jb(w_k, wk, jb)
            dma_w_jb(w_q, wq, jb)
        nc.sync.dma_start(ones8[:], ones_in[:])
        nc.sync.dma_start(ones_bc[:], ones_bc_in[:])
        for kb in range(NKB):
            nc.sync.dma_start(w_v[:, kb, :], wv[kb * KB : (kb + 1) * KB, :])

        # ---- the pipeline ----
        def av_and_drain(d):
            emit_av(d)
            if d.seg_last:
                if d.final:
                    emit_norm(d)
                elif d.tq == 0:
                    emit_spill(d, d.tk < 4)

        for s, d in enumerate(steps):
            if d.tq not in ot_gen:
                ot_gen[d.tq] = [
                    p_ot.tile([KB, TB], BF16, tag="ot", name=f"ot{j}")
                    for j in range(NJB)
                ]
            if s == 0:
                emit_sc(d)
            emit_exp(d)
            for thunk in fillers[s]:
                thunk()
            if s + 1 < len(steps):
                emit_sc(steps[s + 1])
            if s >= AV_LAG:
                av_and_drain(steps[s - AV_LAG])
        for s in range(len(steps) - AV_LAG, len(steps)):
            av_and_drain(steps[s])
        # tail: out projection of the last t-block
        for nb in range(2):
            for ts in range(4):
                out_chain(NTB - 1, nb, ts)()

    nc.compile()
    return nc


def kernel(**inputs: np.ndarray) -> np.ndarray:
    query = np.asarray(inputs["query"], dtype=np.float32)
    key = np.asarray(inputs["key"], dtype=np.float32)
    value = np.asarray(inputs["value"], dtype=np.float32)
    w_q = np.asarray(inputs["w_q"], dtype=np.float32)
    b_q = np.asarray(inputs["b_q"], dtype=np.float32)
    w_k = np.asarray(inputs["w_k"], dtype=np.float32)
    b_k = np.asarray(inputs["b_k"], dtype=np.float32)
    w_v = np.asarray(inputs["w_v"], dtype=np.float32)
    b_v = np.asarray(inputs["b_v"], dtype=np.float32)
    w_o = np.asarray(inputs["w_o"], dtype=np.float32)
    b_o = np.asarray(inputs["b_o"], dtype=np.float32)

    nc = build_kernel()

    bf = ml_dtypes.bfloat16

    def tile_x(a):
        # [T, D] -> transpose -> [kb][tb][128][512] contiguous
        at = a.T.astype(bf)  # [D, T]
        return np.ascontiguousarray(
            at.reshape(NKB, KB, NTB, TB).transpose(0, 2, 1, 3)
        ).reshape(NKB * NTB * KB, TB)

    def tile_w(w_sl):
        # w[sl,:] -> [D, HALF] transposed -> [jb][kb][128][128] contiguous
        wt = w_sl.T.astype(bf)  # [D, HALF]
        return np.ascontiguousarray(
            wt.reshape(NKB, KB, NJB, KB).transpose(2, 0, 1, 3)
        ).reshape(NJB * NKB * KB, KB)

    in_maps = []
    for c in range(N_CORES):
        b = c // 2
        hh = c % 2
        sl = slice(hh * HALF, (hh + 1) * HALF)
        in_maps.append(
            {
                "xq": tile_x(query[b]),
                "xk": tile_x(key[b]),
                "xv": tile_x(value[b]),
                "wq": tile_w(w_q[sl, :]),
                "wk": tile_w(w_k[sl, :]),
                "wv": np.ascontiguousarray(w_v[sl, :].T.astype(bf)),
                "wo": np.ascontiguousarray(w_o[:, sl].T.astype(bf)),
                "bq": np.ascontiguousarray(b_q[sl].reshape(HALF, 1)),
                "bk": np.ascontiguousarray(b_k[sl].reshape(HALF, 1)),
                "ones_in": np.ones((KB, H // 2), dtype=bf),
                "ones_bc_in": np.ones((DH + 1, DH), dtype=bf),
            }
        )

    res = run_bass_kernel_spmd(nc, in_maps, core_ids=list(range(N_CORES)))

    const_row = (b_v[None, :] @ w_o.T + b_o[None, :]).astype(np.float32)
    out = np.empty((B, T, D), dtype=np.float32)
    for b in range(B):
        out[b] = res.results[2 * b]["partial"] + res.results[2 * b + 1]["partial"]
        out[b] += const_row
    return out


# revision 11
# speedup vs baseline: 1.2881x; 1.0983x over previous
"""Multi-head attention (B=4, T=2048, D=1024, H=16) on 8 TRN2 NeuronCores.

Sharding: core c handles batch b = c//2 and head-half hh = c%2 (8 heads,
512 of the 1024 channel dims). Each core computes its half of the head
outputs and a row-sharded output projection, producing a partial
[T, D] output. Host unshard: out[b] = partial[2b] + partial[2b+1]
+ b_o + b_v @ w_o.T (the value-bias contribution commutes through
attention because softmax rows sum to 1).

v8: flat software-pipelined emission, ACT-saturating schedule.
  - The ACT exp stream (256 x [128,1024] activations, ~1.3us each) is the
    critical engine; the driver emits one attention step per exp and
    weaves all projection work into PE slack between steps.
  - Startup critical path: only xk/xq of t-block 0 plus the jb0 weight
    slices (~2.5MB) gate the first exp. Host pre-tiles inputs so every
    DMA source is a contiguous block.
  - tq0's attention is chunked by key-t-block (SBUF accumulation) so it
    streams while K/V of t-blocks 1-3 are still being projected.
  - Scores pairs run concurrently in PE row groups 0/64; softmax
    denominator rides row 64 of the AV accumulators (ones column in V);
    a K=1 PE matmul broadcasts it across partitions.
"""

from contextlib import ExitStack

import numpy as np
import ml_dtypes

import concourse.bass as bass
import concourse.mybir as mybir
import concourse.tile as tile
from concourse import bacc
from concourse.bass_utils import run_bass_kernel_spmd

B, T, D = 4, 2048, 1024
H = 16
DH = 64  # head dim
HALF = 512  # channels per core (8 heads)
N_CORES = 8

F32 = mybir.dt.float32
BF16 = mybir.dt.bfloat16

TB = 512  # t-block for moving operands
NTB = T // TB  # 4
KB = 128  # contraction block
NKB = D // KB  # 8
NJB = HALF // KB  # 4 j-blocks of the half
NTK = T // KB  # 16 tk blocks


class Step:
    __slots__ = ("tq", "jp", "tk", "seg_first", "seg_last", "final", "sc", "ex", "seg")

    def __init__(self, tq, jp, tk, seg_first, seg_last, final):
        self.tq = tq
        self.jp = jp
        self.tk = tk
        self.seg_first = seg_first  # first step of an av accumulation segment
        self.seg_last = seg_last  # last step of an av accumulation segment
        self.final = final  # last segment of this (tq, jp): normalize after
        self.sc = None
        self.ex = None
        self.seg = None


def build_kernel():
    nc = bacc.Bacc(
        "TRN2", target_bir_lowering=False, debug=False, num_devices=N_CORES
    )
    # pre-tiled inputs: x*[kb][tb] -> [128, 512] contiguous blocks
    xq = nc.dram_tensor("xq", [NKB * NTB * KB, TB], BF16, kind="ExternalInput").ap()
    xk = nc.dram_tensor("xk", [NKB * NTB * KB, TB], BF16, kind="ExternalInput").ap()
    xv = nc.dram_tensor("xv", [NKB * NTB * KB, TB], BF16, kind="ExternalInput").ap()
    # wq/wk tiled [jb][kb] -> [128, 128] contiguous blocks
    wq = nc.dram_tensor("wq", [NJB * NKB * KB, KB], BF16, kind="ExternalInput").ap()
    wk = nc.dram_tensor("wk", [NJB * NKB * KB, KB], BF16, kind="ExternalInput").ap()
    # wv rows contiguous per kb block; wo rows contiguous per jb block
    wv = nc.dram_tensor("wv", [NKB * KB, HALF], BF16, kind="ExternalInput").ap()
    wo = nc.dram_tensor("wo", [HALF, D], BF16, kind="ExternalInput").ap()
    bq = nc.dram_tensor("bq", [HALF, 1], F32, kind="ExternalInput").ap()
    bk = nc.dram_tensor("bk", [HALF, 1], F32, kind="ExternalInput").ap()
    ones_in = nc.dram_tensor("ones_in", [KB, H // 2], BF16, kind="ExternalInput").ap()
    ones_bc_in = nc.dram_tensor(
        "ones_bc_in", [DH + 1, DH], BF16, kind="ExternalInput"
    ).ap()
    partial = nc.dram_tensor("partial", [T, D], F32, kind="ExternalOutput").ap()

    with tile.TileContext(nc) as tc, ExitStack() as ctx:
        p_const = ctx.enter_context(tc.tile_pool(name="const", bufs=1))
        p_kt = ctx.enter_context(tc.tile_pool(name="kt", bufs=NJB * NTB))
        p_v = ctx.enter_context(tc.tile_pool(name="v", bufs=NTK))
        p_qt = ctx.enter_context(tc.tile_pool(name="qt", bufs=2 * NJB))
        p_xs = ctx.enter_context(tc.tile_pool(name="xs", bufs=5))
        p_ex = ctx.enter_context(tc.tile_pool(name="ex", bufs=8))
        p_ot = ctx.enter_context(tc.tile_pool(name="ot", bufs=2 * NJB))
        p_as = ctx.enter_context(tc.tile_pool(name="as", bufs=6))
        p_ac = ctx.enter_context(tc.tile_pool(name="ac", bufs=8))
        p_rc = ctx.enter_context(tc.tile_pool(name="rc", bufs=3))
        p_st = ctx.enter_context(tc.tile_pool(name="st", bufs=2))
        # PSUM: scores 2x[128,1024] (4 banks) + av 2x[65,512] + pj 2x[128,512]
        p_sc = ctx.enter_context(tc.tile_pool(name="sc", bufs=2, space="PSUM"))
        p_av = ctx.enter_context(tc.tile_pool(name="av", bufs=2, space="PSUM"))
        p_pj = ctx.enter_context(tc.tile_pool(name="pj", bufs=2, space="PSUM"))

        # ---- persistent SBUF tiles ----
        w_k = p_const.tile([KB, NKB, HALF], BF16, tag="wk")
        w_q = p_const.tile([KB, NKB, HALF], BF16, tag="wq")
        w_v = p_const.tile([KB, NKB, HALF], BF16, tag="wv")
        w_o = p_const.tile([KB, NJB, D], BF16, tag="wo")
        b_k = p_const.tile([KB, NJB], F32, tag="bk")
        b_q = p_const.tile([KB, NJB], F32, tag="bq")
        ones8 = p_const.tile([KB, H // 2], BF16, tag="ones8")
        ones_bc = p_const.tile([DH + 1, DH], BF16, tag="onesbc")
        warm = p_const.tile([1, 8], F32, tag="warm")
        warm_o = p_const.tile([1, 8], BF16, tag="warmo")

        kt_tiles = [
            [p_kt.tile([KB, TB], BF16, tag="kt", name=f"kt{j}_{tb}") for tb in range(NTB)]
            for j in range(NJB)
        ]
        v_tiles = [
            p_v.tile([KB, H // 2, DH + 1], BF16, tag="v", name=f"v{j}")
            for j in range(NTK)
        ]

        # ---- DMA emitters ----
        def dma_w_jb(dst, src, jb):
            # one jb slice of wq/wk in a single DMA: [128, kb 8, 128]
            nc.sync.dma_start(
                dst[:, :, jb * KB : (jb + 1) * KB],
                src[jb * NKB * KB : (jb + 1) * NKB * KB, :].rearrange(
                    "(kb p) j -> p kb j", p=KB
                ),
            )

        def load_x_tile(src, tb):
            xt = p_xs.tile([KB, NKB, TB], BF16, tag="xs")
            nc.sync.dma_start(
                xt[:],
                src[tb * NKB * KB : (tb + 1) * NKB * KB, :].rearrange(
                    "(kb p) t -> p kb t", p=KB
                ),
            )
            return xt

        # ---- projection chain emitters (each returns two half-chain thunks
        # so the driver can spread chains across pipeline steps) ----
        xk_tiles = {}  # tb -> tiles (shared by the 4 jb chains)
        xq_tiles = {}
        xv_tiles = {}
        HC = NKB // 2  # MMs per half-chain

        def k_parts(tb, jb):
            st = {}

            def a():
                if tb not in xk_tiles:
                    xk_tiles[tb] = load_x_tile(xk, tb)
                st["ps"] = p_pj.tile([KB, TB], F32, tag="pj", name="ps")
                for kb in range(HC):
                    nc.tensor.matmul(
                        st["ps"][:],
                        w_k[:, kb, jb * KB : (jb + 1) * KB],
                        xk_tiles[tb][:, kb, :],
                        start=(kb == 0),
                        stop=False,
                    )

            def b():
                for kb in range(HC, NKB):
                    nc.tensor.matmul(
                        st["ps"][:],
                        w_k[:, kb, jb * KB : (jb + 1) * KB],
                        xk_tiles[tb][:, kb, :],
                        start=False,
                        stop=(kb == NKB - 1),
                    )
                nc.vector.tensor_scalar_add(
                    kt_tiles[jb][tb][:], st["ps"][:], b_k[:, jb : jb + 1]
                )
                if jb == NJB - 1:
                    del xk_tiles[tb]

            return a, b

        def v_parts(tb, ts):
            st = {}

            def a():
                if tb not in xv_tiles:
                    for u in range(4):
                        nc.vector.tensor_copy(
                            v_tiles[tb * 4 + u][:, :, DH : DH + 1],
                            ones8[:, :, None],
                        )
                    xv_tiles[tb] = load_x_tile(xv, tb)
                st["ps"] = p_pj.tile([KB, TB], F32, tag="pj", name="ps")
                for kb in range(HC):
                    nc.tensor.matmul(
                        st["ps"][:],
                        xv_tiles[tb][:, kb, ts * KB : (ts + 1) * KB],
                        w_v[:, kb, :],
                        start=(kb == 0),
                        stop=False,
                    )

            def b():
                for kb in range(HC, NKB):
                    nc.tensor.matmul(
                        st["ps"][:],
                        xv_tiles[tb][:, kb, ts * KB : (ts + 1) * KB],
                        w_v[:, kb, :],
                        start=False,
                        stop=(kb == NKB - 1),
                    )
                nc.vector.tensor_copy(
                    v_tiles[tb * 4 + ts][:, :, 0:DH],
                    st["ps"][:].rearrange("p (h d) -> p h d", d=DH),
                )
                if ts == 3:
                    del xv_tiles[tb]

            return a, b

        qt_gen = {}  # tq -> list of qt tiles

        def q_parts(tq, jb):
            st = {}

            def a():
                if tq not in qt_gen:
                    qt_gen[tq] = [None] * NJB
                    xq_tiles[tq] = load_x_tile(xq, tq)
                st["ps"] = p_pj.tile([KB, TB], F32, tag="pj", name="ps")
                for kb in range(HC):
                    nc.tensor.matmul(
                        st["ps"][:],
                        w_q[:, kb, jb * KB : (jb + 1) * KB],
                        xq_tiles[tq][:, kb, :],
                        start=(kb == 0),
                        stop=False,
                    )

            def b():
                for kb in range(HC, NKB):
                    nc.tensor.matmul(
                        st["ps"][:],
                        w_q[:, kb, jb * KB : (jb + 1) * KB],
                        xq_tiles[tq][:, kb, :],
                        start=False,
                        stop=(kb == NKB - 1),
                    )
                qt = p_qt.tile([KB, TB], BF16, tag="qt", name=f"qt{jb}")
                nc.vector.tensor_scalar_add(qt[:], st["ps"][:], b_q[:, jb : jb + 1])
                qt_gen[tq][jb] = qt
                if jb == NJB - 1:
                    del xq_tiles[tq]

            return a, b

        # ---- attention step emitters ----
        def emit_sc(d):
            d.sc = p_sc.tile([KB, 2 * TB], F32, tag="sc", name="sc")
            qt = qt_gen[d.tq]
            for i in range(2):
                nc.tensor.matmul(
                    d.sc[:, i * TB : (i + 1) * TB],
                    kt_tiles[d.jp][d.tk // 4][
                        i * DH : (i + 1) * DH,
                        (d.tk % 4) * KB : (d.tk % 4 + 1) * KB,
                    ],
                    qt[d.jp][i * DH : (i + 1) * DH, :],
                    start=True,
                    stop=True,
                )

        def emit_exp(d):
            d.ex = p_ex.tile([KB, 2 * TB], BF16, tag="ex", name="ex")
            nc.scalar.activation(
                d.ex[:], d.sc[:], mybir.ActivationFunctionType.Exp, scale=0.125
            )

        seg_avs = {}  # (tq, jp) -> current av psum pair

        def emit_av(d):
            if d.seg_first:
                seg_avs[d.tq, d.jp] = [
                    p_av.tile([DH + 1, TB], F32, tag="av", name=f"av{i}")
                    for i in range(2)
                ]
            avs = seg_avs[d.tq, d.jp]
            for i in range(2):
                nc.tensor.matmul(
                    avs[i][:],
                    v_tiles[d.tk][:, 2 * d.jp + i, :],
                    d.ex[:, i * TB : (i + 1) * TB],
                    start=d.seg_first,
                    stop=d.seg_last,
                )

        acc = {}  # (tq, jp, i) -> sbuf accumulator (chunked tq only)
        ot_gen = {}  # tq -> ot tiles

        def emit_spill(d, first_chunk):
            """Chunked path: move/add av psum pair into SBUF accumulators."""
            avs = seg_avs.pop((d.tq, d.jp))
            for i in range(2):
                if first_chunk:
                    acc[d.tq, d.jp, i] = p_ac.tile(
                        [DH + 1, TB], F32, tag="ac", name=f"ac{d.jp}_{i}"
                    )
                    nc.vector.tensor_copy(acc[d.tq, d.jp, i][:], avs[i][:])
                else:
                    nc.vector.tensor_add(
                        acc[d.tq, d.jp, i][:], acc[d.tq, d.jp, i][:], avs[i][:]
                    )

        def normalize(tq, jp, i, src):
            """src: [65, TB] f32 head-pair output; row 64 = denominator."""
            ot_tiles = ot_gen[tq]
            db = p_as.tile([DH + 1, TB], BF16, tag="db")
            nc.vector.tensor_copy(db[DH : DH + 1, :], src[DH : DH + 1, :])
            dbc = p_av.tile([DH, TB], F32, tag="av", name="dbc")
            nc.tensor.matmul(
                dbc[:],
                ones_bc[DH : DH + 1, :],
                db[DH : DH + 1, :],
                start=True,
                stop=True,
            )
            rc2 = p_rc.tile([DH, TB], F32, tag="rc2")
            nc.vector.reciprocal_approx_fast(rc2[:], dbc[:])
            if i == 0:
                nc.vector.tensor_mul(ot_tiles[jp][0:DH, :], src[0:DH, :], rc2[:])
            else:
                stg = p_rc.tile([DH, TB], BF16, tag="stg")
                nc.vector.tensor_mul(stg[:], src[0:DH, :], rc2[:])
                nc.sync.dma_start(ot_tiles[jp][DH : 2 * DH, :], stg[:])

        def emit_norm(d):
            """Final segment of (tq, jp): drain av psum / acc and normalize."""
            if d.tq == 0:
                emit_spill(d, False)
                srcs = [acc[d.tq, d.jp, i] for i in range(2)]
            else:
                avs = seg_avs.pop((d.tq, d.jp))
                srcs = []
                for i in range(2):
                    a = p_as.tile([DH + 1, TB], F32, tag="as")
                    nc.vector.tensor_copy(a[:], avs[i][:])
                    srcs.append(a)
            for i in (1, 0):
                normalize(d.tq, d.jp, i, srcs[i])

        def out_chain(tq, nb, ts):
            def emit():
                ot_tiles = ot_gen[tq]
                po = p_pj.tile([KB, TB], F32, tag="pj")
                for jp in range(NJB):
                    nc.tensor.matmul(
                        po[:],
                        ot_tiles[jp][:, ts * KB : (ts + 1) * KB],
                        w_o[:, jp, nb * TB : (nb + 1) * TB],
                        start=(jp == 0),
                        stop=(jp == NJB - 1),
                    )
                st = p_st.tile([KB, TB], F32, tag="st")
                nc.vector.tensor_copy(st[:], po[:])
                nc.sync.dma_start(
                    partial[
                        tq * TB + ts * KB : tq * TB + (ts + 1) * KB,
                        nb * TB : (nb + 1) * TB,
                    ],
                    st[:],
                )

            return emit

        # ---- build the global step list ----
        steps = []
        for tq in range(NTB):
            if tq == 0:
                for ci in range(4):
                    for jp in range(NJB):
                        for u in range(4):
                            steps.append(
                                Step(tq, jp, ci * 4 + u, u == 0, u == 3, ci == 3)
                            )
            else:
                for jp in range(NJB):
                    for u in range(NTK):
                        steps.append(Step(tq, jp, u, u == 0, u == NTK - 1, True))

        # ---- per-step filler plan: fillers[s] emitted between exp(s) and
        # sc(s+1).  Deadlines: a chain feeding sc(s+1) must be in fillers[<=s];
        # one feeding av(s-LAG) must be in fillers[<=s] as well (av comes last).
        AV_LAG = 6
        fillers = [[] for _ in steps]

        def place(s, thunk):
            fillers[min(max(s, 0), len(steps) - 1)].append(thunk)

        # tq0: K/Q chains jb1-3 (sc(4*jb) lookahead at step 4*jb-1)
        for j, s0 in [(1, 0), (2, 4), (3, 8)]:
            ka, kb_ = k_parts(0, j)
            qa, qb = q_parts(0, j)
            place(s0, ka)
            place(s0 + 1, kb_)
            place(s0 + 2, qa)
            place(s0 + 3, qb)
        # V tb0: av(u) runs at step u+AV_LAG
        for u in range(4):
            va, vb = v_parts(0, u)
            place(2 + 2 * u, va)
            place(3 + 2 * u, vb)
        # K/V of tb 1-3: kt[jp][tb] first read by sc at step 16*tb+4*jp-1;
        # v_tiles[4*tb+u] first read by av at step 16*tb+u+AV_LAG.
        for tb in range(1, NTB):
            for j in range(NJB):
                ka, kb_ = k_parts(tb, j)
                place(16 * tb - 6 + 2 * j, ka)
                place(16 * tb - 5 + 2 * j, kb_)
            for u in range(4):
                va, vb = v_parts(tb, u)
                place(16 * tb + 2 + 2 * u, va)
                place(16 * tb + 3 + 2 * u, vb)
        place(30, lambda: nc.sync.dma_start(
            w_o[:], wo.rearrange("(jb p) n -> p jb n", p=KB)))
        # q_proj(tq+1) near the end of tq; out_proj(tq-1) spread across tq.
        for tq in range(NTB):
            base = tq * 64
            if tq + 1 < NTB:
                for jb in range(NJB):
                    qa, qb = q_parts(tq + 1, jb)
                    place(base + 56 + 2 * jb, qa)
                    place(base + 57 + 2 * jb, qb)
            if tq > 0:
                chains = [out_chain(tq - 1, nb, ts) for nb in range(2) for ts in range(4)]
                for idx, ch in enumerate(chains):
                    place(base + 6 + 4 * idx, ch)

        # ---- emission ----
        # ACT table pre-warm: a tiny exp long before the first real one
        nc.sync.dma_start(warm[:, 0:1], bq[0:1, 0:1])
        nc.scalar.activation(
            warm_o[:, 0:1], warm[:, 0:1], mybir.ActivationFunctionType.Exp, scale=1.0
        )

        # startup DMA critical path
        dma_w_jb(w_k, wk, 0)
        nc.sync.dma_start(b_k[:], bk.rearrange("(jb p) one -> p (jb one)", p=KB))
        ka0, kb0 = k_parts(0, 0)
        ka0()
        kb0()
        dma_w_jb(w_q, wq, 0)
        nc.sync.dma_start(b_q[:], bq.rearrange("(jb p) one -> p (jb one)", p=KB))
        qa0, qb0 = q_parts(0, 0)
        qa0()
        qb0()
        # next DMAs: remaining wk/wq jb slices, then V-path constants
        for jb in range(1, NJB):
            dma_w_jb(w_k, wk, jb)
            dma_w_jb(w_q, wq, jb)
        nc.sync.dma_start(ones8[:], ones_in[:])
        nc.sync.dma_start(ones_bc[:], ones_bc_in[:])
        nc.sync.dma_start(
            w_v[:], wv.rearrange("(kb p) n -> p kb n", p=KB)
        )

        # ---- the pipeline ----
        def av_and_drain(d):
            emit_av(d)
            if d.seg_last:
                if d.final:
                    emit_norm(d)
                elif d.tq == 0:
                    emit_spill(d, d.tk < 4)

        for s, d in enumerate(steps):
            if d.tq not in ot_gen:
                ot_gen[d.tq] = [
                    p_ot.tile([KB, TB], BF16, tag="ot", name=f"ot{j}")
                    for j in range(NJB)
                ]
            if s == 0:
                emit_sc(d)
            emit_exp(d)
            for thunk in fillers[s]:
                thunk()
            if s + 1 < len(steps):
                emit_sc(steps[s + 1])
            if s >= AV_LAG:
                av_and_drain(steps[s - AV_LAG])
        for s in range(len(steps) - AV_LAG, len(steps)):
            av_and_drain(steps[s])
        # tail: out projection of the last t-block
        for nb in range(2):
            for ts in range(4):
                out_chain(NTB - 1, nb, ts)()

    nc.compile()
    return nc


def kernel(**inputs: np.ndarray) -> np.ndarray:
    query = np.asarray(inputs["query"], dtype=np.float32)
    key = np.asarray(inputs["key"], dtype=np.float32)
    value = np.asarray(inputs["value"], dtype=np.float32)
    w_q = np.asarray(inputs["w_q"], dtype=np.float32)
    b_q = np.asarray(inputs["b_q"], dtype=np.float32)
    w_k = np.asarray(inputs["w_k"], dtype=np.float32)
    b_k = np.asarray(inputs["b_k"], dtype=np.float32)
    w_v = np.asarray(inputs["w_v"], dtype=np.float32)
    b_v = np.asarray(inputs["b_v"], dtype=np.float32)
    w_o = np.asarray(inputs["w_o"], dtype=np.float32)
    b_o = np.asarray(inputs["b_o"], dtype=np.float32)

    nc = build_kernel()

    bf = ml_dtypes.bfloat16

    def tile_x(a):
        # [T, D] -> transpose -> [kb][tb][128][512] contiguous
        at = a.T.astype(bf)  # [D, T]
        return np.ascontiguousarray(
            at.reshape(NKB, KB, NTB, TB).transpose(2, 0, 1, 3)
        ).reshape(NKB * NTB * KB, TB)

    def tile_w(w_sl):
        # w[sl,:] -> [D, HALF] transposed -> [jb][kb][128][128] contiguous
        wt = w_sl.T.astype(bf)  # [D, HALF]
        return np.ascontiguousarray(
            wt.reshape(NKB, KB, NJB, KB).transpose(2, 0, 1, 3)
        ).reshape(NJB * NKB * KB, KB)

    in_maps = []
    for c in range(N_CORES):
        b = c // 2
        hh = c % 2
        sl = slice(hh * HALF, (hh + 1) * HALF)
        in_maps.append(
            {
                "xq": tile_x(query[b]),
                "xk": tile_x(key[b]),
                "xv": tile_x(value[b]),
                "wq": tile_w(w_q[sl, :]),
                "wk": tile_w(w_k[sl, :]),
                "wv": np.ascontiguousarray(w_v[sl, :].T.astype(bf)),
                "wo": np.ascontiguousarray(w_o[:, sl].T.astype(bf)),
                "bq": np.ascontiguousarray(b_q[sl].reshape(HALF, 1)),
                "bk": np.ascontiguousarray(b_k[sl].reshape(HALF, 1)),
                "ones_in": np.ones((KB, H // 2), dtype=bf),
                "ones_bc_in": np.ones((DH + 1, DH), dtype=bf),
            }
        )

    res = run_bass_kernel_spmd(nc, in_maps, core_ids=list(range(N_CORES)))

    const_row = (b_v[None, :] @ w_o.T + b_o[None, :]).astype(np.float32)
    out = np.empty((B, T, D), dtype=np.float32)
    for b in range(B):
        out[b] = res.results[2 * b]["partial"] + res.results[2 * b + 1]["partial"]
        out[b] += const_row
    return out
